# revision 3
# baseline (speedup 1.0000x reference)
"""Trainium2 Bass kernel for nn_CMFA (dense_transformer, seq_len=1 cross-attention).

Math notes (exact simplifications vs the reference):
  - softmax over a single key is exactly 1.0, so the attention output is
    exactly the v-projection: mha(q,k,v) = (v @ Wv.T + bv) @ Wo.T + bo.
    The q/k projections never influence the output.
  - Wv -> Wo -> fi2 is a linear chain (no nonlinearity), so it is folded on
    the host:  V = [v1, i_] @ Wcat.T + bcat  with
      Wcat = [fi2 @ (Wo @ Wv), fi2],  bcat = fi2 @ (Wo @ bv + bo) + fi2_b
    (the i_ column block carries the residual through fi2).

Device layout: activations are feature-major ("transposed", [feat, batch]) so
every matmul contracts over the partition dim and every DMA is contiguous.
The host pre-transposes the batch shards of i/t and transposes the output
back. Pure data parallel across 8 cores; weights replicated.
"""

import numpy as np

B, IMG, TAB, HID = 32768, 2048, 128, 512
NCORES = 8
BS = B // NCORES  # rows per core
NT = 512          # batch-tile (matmul moving/free dim)

_CACHE = {}


def _pack_blocks(WT: np.ndarray, K: int, M: int) -> np.ndarray:
    """[K*128, M*128] -> [128, K*M*128] with col ((k*M+m)*128 + j) = WT[k*128+p, m*128+j]."""
    out = WT.reshape(K, 128, M, 128).transpose(1, 0, 2, 3).reshape(128, K * M * 128)
    return np.ascontiguousarray(out, dtype=np.float32)


def _build_nc(bs: int):
    import concourse.bass as bass
    import concourse.tile as tile
    from concourse import bacc, mybir

    f32 = mybir.dt.float32
    f32r = mybir.dt.float32r
    Relu = mybir.ActivationFunctionType.Relu
    Ident = mybir.ActivationFunctionType.Identity
    ntiles = bs // NT

    nc = bacc.Bacc("TRN2", target_bir_lowering=False, debug=False)

    iT_d = nc.dram_tensor("iT", [IMG, bs], f32r, kind="ExternalInput").ap()
    tT_d = nc.dram_tensor("tT", [TAB, bs], f32r, kind="ExternalInput").ap()
    w_fi1_d = nc.dram_tensor("w_fi1", [128, 64 * 128], f32r, kind="ExternalInput").ap()
    w_ft1_d = nc.dram_tensor("w_ft1", [128, 4 * 128], f32r, kind="ExternalInput").ap()
    w_ci1_d = nc.dram_tensor("w_ci1", [128, 16 * 128], f32r, kind="ExternalInput").ap()
    w_ct1_d = nc.dram_tensor("w_ct1", [128, 16 * 128], f32r, kind="ExternalInput").ap()
    w_V_d = nc.dram_tensor("w_V", [128, 32 * 128], f32r, kind="ExternalInput").ap()
    w_T_d = nc.dram_tensor("w_T", [128, 32 * 128], f32r, kind="ExternalInput").ap()
    bias_d = nc.dram_tensor("bias", [128, 24], f32, kind="ExternalInput").ap()
    out_d = nc.dram_tensor("outT", [2 * HID, bs], f32, kind="ExternalOutput").ap()

    with tile.TileContext(nc) as tc:
        with (
            tc.tile_pool(name="w", bufs=1) as wpool,
            tc.tile_pool(name="x", bufs=6) as xpool,
            tc.tile_pool(name="h", bufs=8) as hpool,
            tc.tile_pool(name="o", bufs=12) as opool,
            tc.tile_pool(name="ps", bufs=8, space="PSUM") as pspool,
        ):
            wf1 = wpool.tile([128, 64 * 128], f32r)
            nc.sync.dma_start(wf1[:], w_fi1_d[:])
            wt1 = wpool.tile([128, 4 * 128], f32r)
            nc.sync.dma_start(wt1[:], w_ft1_d[:])
            wc1 = wpool.tile([128, 16 * 128], f32r)
            nc.sync.dma_start(wc1[:], w_ci1_d[:])
            wc2 = wpool.tile([128, 16 * 128], f32r)
            nc.sync.dma_start(wc2[:], w_ct1_d[:])
            wV = wpool.tile([128, 32 * 128], f32r)
            nc.sync.dma_start(wV[:], w_V_d[:])
            wT = wpool.tile([128, 32 * 128], f32r)
            nc.sync.dma_start(wT[:], w_T_d[:])
            bt = wpool.tile([128, 24], f32)
            nc.sync.dma_start(bt[:], bias_d[:])

            def mm(ps_ap, wtile, blk, x_ap, start, stop):
                nc.tensor.matmul(
                    ps_ap,
                    wtile[:, blk * 128:(blk + 1) * 128],
                    x_ap,
                    start=start,
                    stop=stop,
                )

            for n in range(ntiles):
                c0 = n * NT
                # ---- i_ = relu(i @ fi1.T + b) ----
                ps = [pspool.tile([128, NT], f32, tag="ps", name=f"ps1_{n}_{_m}") for _m in range(4)]
                for k in range(16):
                    xk = xpool.tile([128, NT], f32r, tag="x", name=f"xk_{n}_{k}")
                    nc.sync.dma_start(xk[:], iT_d[128 * k:128 * (k + 1), c0:c0 + NT])
                    for m in range(4):
                        mm(ps[m][:], wf1, k * 4 + m, xk[:], k == 0, k == 15)
                i_ = [hpool.tile([128, NT], f32r, tag="i_", name=f"i__{n}_{_m}") for _m in range(4)]
                for m in range(4):
                    nc.scalar.activation(i_[m][:], ps[m][:], Relu, bias=bt[:, m:m + 1])

                # ---- t_ = relu(t @ ft1.T + b) ----
                xt = xpool.tile([128, NT], f32r, tag="xt", bufs=2, name=f"xt_{n}")
                nc.sync.dma_start(xt[:], tT_d[:, c0:c0 + NT])
                ps2 = [pspool.tile([128, NT], f32, tag="ps", name=f"ps2_{n}_{_m}") for _m in range(4)]
                for m in range(4):
                    mm(ps2[m][:], wt1, m, xt[:], True, True)
                t_ = [hpool.tile([128, NT], f32r, tag="t_", name=f"t__{n}_{_m}") for _m in range(4)]
                for m in range(4):
                    nc.scalar.activation(t_[m][:], ps2[m][:], Relu, bias=bt[:, 4 + m:5 + m])

                # ---- v1 = relu(i_ @ ci1.T + b) ----
                ps3 = [pspool.tile([128, NT], f32, tag="ps", name=f"ps3_{n}_{_m}") for _m in range(4)]
                for k in range(4):
                    for m in range(4):
                        mm(ps3[m][:], wc1, k * 4 + m, i_[k][:], k == 0, k == 3)
                v1 = [hpool.tile([128, NT], f32r, tag="v1", name=f"v1_{n}_{_m}") for _m in range(4)]
                for m in range(4):
                    nc.scalar.activation(v1[m][:], ps3[m][:], Relu, bias=bt[:, 8 + m:9 + m])

                # ---- v2 = relu(t_ @ ct1.T + b) ----
                ps4 = [pspool.tile([128, NT], f32, tag="ps", name=f"ps4_{n}_{_m}") for _m in range(4)]
                for k in range(4):
                    for m in range(4):
                        mm(ps4[m][:], wc2, k * 4 + m, t_[k][:], k == 0, k == 3)
                v2 = [hpool.tile([128, NT], f32r, tag="v2", name=f"v2_{n}_{_m}") for _m in range(4)]
                for m in range(4):
                    nc.scalar.activation(v2[m][:], ps4[m][:], Relu, bias=bt[:, 12 + m:13 + m])

                # ---- V = [v1, i_] @ WcatV.T + bcatV ----
                psV = [pspool.tile([128, NT], f32, tag="ps", name=f"psV_{n}_{_m}") for _m in range(4)]
                for k in range(4):
                    for m in range(4):
                        mm(psV[m][:], wV, k * 4 + m, v1[k][:], k == 0, False)
                for k in range(4):
                    for m in range(4):
                        mm(psV[m][:], wV, (4 + k) * 4 + m, i_[k][:], False, k == 3)
                for m in range(4):
                    oV = opool.tile([128, NT], f32, tag="o", name=f"oV_{n}_{m}")
                    nc.scalar.activation(oV[:], psV[m][:], Ident, bias=bt[:, 16 + m:17 + m])
                    nc.sync.dma_start(out_d[128 * m:128 * (m + 1), c0:c0 + NT], oV[:])

                # ---- T = [v2, t_] @ WcatT.T + bcatT ----
                psT = [pspool.tile([128, NT], f32, tag="ps", name=f"psT_{n}_{_m}") for _m in range(4)]
                for k in range(4):
                    for m in range(4):
                        mm(psT[m][:], wT, k * 4 + m, v2[k][:], k == 0, False)
                for k in range(4):
                    for m in range(4):
                        mm(psT[m][:], wT, (4 + k) * 4 + m, t_[k][:], False, k == 3)
                for m in range(4):
                    oT = opool.tile([128, NT], f32, tag="o", name=f"oT_{n}_{m}")
                    nc.scalar.activation(oT[:], psT[m][:], Ident, bias=bt[:, 20 + m:21 + m])
                    nc.sync.dma_start(
                        out_d[HID + 128 * m:HID + 128 * (m + 1), c0:c0 + NT], oT[:]
                    )

    nc.compile()
    return nc


def _host_pack(inp: dict):
    f8 = np.float64
    fi1_w, fi1_b = inp["fi1_w"], inp["fi1_b"]
    ft1_w, ft1_b = inp["ft1_w"], inp["ft1_b"]
    ci1_w, ci1_b = inp["ci1_w"], inp["ci1_b"]
    ct1_w, ct1_b = inp["ct1_w"], inp["ct1_b"]

    def fold(wv, bv, wo, bo, f_w, f_b):
        Wvo = wo.astype(f8) @ wv.astype(f8)
        bvo = wo.astype(f8) @ bv.astype(f8) + bo.astype(f8)
        Wcat = np.concatenate([f_w.astype(f8) @ Wvo, f_w.astype(f8)], axis=1)
        bcat = f_w.astype(f8) @ bvo + f_b.astype(f8)
        return Wcat.astype(np.float32), bcat.astype(np.float32)

    WcatV, bcatV = fold(inp["aV_wv"], inp["aV_bv"], inp["aV_wo"], inp["aV_bo"],
                        inp["fi2_w"], inp["fi2_b"])
    WcatT, bcatT = fold(inp["aT_wv"], inp["aT_bv"], inp["aT_wo"], inp["aT_bo"],
                        inp["ft2_w"], inp["ft2_b"])

    weights = {
        "w_fi1": _pack_blocks(np.ascontiguousarray(fi1_w.T), 16, 4),
        "w_ft1": _pack_blocks(np.ascontiguousarray(ft1_w.T), 1, 4),
        "w_ci1": _pack_blocks(np.ascontiguousarray(ci1_w.T), 4, 4),
        "w_ct1": _pack_blocks(np.ascontiguousarray(ct1_w.T), 4, 4),
        "w_V": _pack_blocks(np.ascontiguousarray(WcatV.T), 8, 4),
        "w_T": _pack_blocks(np.ascontiguousarray(WcatT.T), 8, 4),
    }
    cols = []
    for b in (fi1_b, ft1_b, ci1_b, ct1_b, bcatV, bcatT):
        for m in range(4):
            cols.append(b[128 * m:128 * (m + 1)])
    weights["bias"] = np.ascontiguousarray(np.stack(cols, axis=1), dtype=np.float32)
    return weights


def kernel(**inputs) -> np.ndarray:
    from concourse import bass_utils

    i = np.asarray(inputs["i"], dtype=np.float32)
    t = np.asarray(inputs["t"], dtype=np.float32)
    weights = _host_pack(inputs)

    if "nc" not in _CACHE:
        _CACHE["nc"] = _build_nc(BS)
    nc = _CACHE["nc"]

    in_maps = []
    for c in range(NCORES):
        sl = slice(c * BS, (c + 1) * BS)
        m = dict(weights)
        m["iT"] = np.ascontiguousarray(i[sl].T)
        m["tT"] = np.ascontiguousarray(t[sl].T)
        in_maps.append(m)

    res = bass_utils.run_bass_kernel_spmd(nc, in_maps, core_ids=list(range(NCORES)))

    out = np.empty((B, 2 * HID), dtype=np.float32)
    for c in range(NCORES):
        out[c * BS:(c + 1) * BS] = res.results[c]["outT"].T
    return out


# revision 4
# speedup vs baseline: 1.0245x; 1.0245x over previous
"""Trainium2 Bass kernel for nn_CMFA (dense_transformer, seq_len=1 cross-attention).

Math notes (exact simplifications vs the reference):
  - softmax over a single key is exactly 1.0, so the attention output is
    exactly the v-projection: mha(q,k,v) = (v @ Wv.T + bv) @ Wo.T + bo.
    The q/k projections never influence the output.
  - Wv -> Wo -> fi2 is a linear chain (no nonlinearity), so it is folded on
    the host:  V = [v1, i_] @ Wcat.T + bcat  with
      Wcat = [fi2 @ (Wo @ Wv), fi2],  bcat = fi2 @ (Wo @ bv + bo) + fi2_b
    (the i_ column block carries the residual through fi2).

Device layout: activations are feature-major ("transposed", [feat, batch]) so
every matmul contracts over the partition dim and every DMA is contiguous.
The host pre-transposes the batch shards of i/t and transposes the output
back. Pure data parallel across 8 cores; weights replicated.

Weights are loaded as one SBUF tile per (layer, k-chunk) so the first fi1
matmuls only depend on their own 256KB chunk, not the whole 10.7MB weight
set; the first batch-tile's input DMAs are interleaved with the weight
chunks so the PE starts within a few microseconds.
"""

import numpy as np

B, IMG, TAB, HID = 32768, 2048, 128, 512
NCORES = 8
BS = B // NCORES  # rows per core
NT = 512          # batch-tile (matmul moving/free dim)

_CACHE = {}


def _pack_blocks(WT: np.ndarray, K: int, M: int) -> np.ndarray:
    """[K*128, M*128] -> [128, K*M*128] with col ((k*M+m)*128 + j) = WT[k*128+p, m*128+j]."""
    out = WT.reshape(K, 128, M, 128).transpose(1, 0, 2, 3).reshape(128, K * M * 128)
    return np.ascontiguousarray(out, dtype=np.float32)


def _build_nc(bs: int):
    import concourse.bass as bass
    import concourse.tile as tile
    from concourse import bacc, mybir

    f32 = mybir.dt.float32
    f32r = mybir.dt.float32r
    Relu = mybir.ActivationFunctionType.Relu
    Ident = mybir.ActivationFunctionType.Identity
    ntiles = bs // NT

    nc = bacc.Bacc("TRN2", target_bir_lowering=False, debug=False)

    iT_d = nc.dram_tensor("iT", [IMG, bs], f32r, kind="ExternalInput").ap()
    tT_d = nc.dram_tensor("tT", [TAB, bs], f32r, kind="ExternalInput").ap()
    w_fi1_d = nc.dram_tensor("w_fi1", [128, 64 * 128], f32r, kind="ExternalInput").ap()
    w_ft1_d = nc.dram_tensor("w_ft1", [128, 4 * 128], f32r, kind="ExternalInput").ap()
    w_ci1_d = nc.dram_tensor("w_ci1", [128, 16 * 128], f32r, kind="ExternalInput").ap()
    w_ct1_d = nc.dram_tensor("w_ct1", [128, 16 * 128], f32r, kind="ExternalInput").ap()
    w_V_d = nc.dram_tensor("w_V", [128, 32 * 128], f32r, kind="ExternalInput").ap()
    w_T_d = nc.dram_tensor("w_T", [128, 32 * 128], f32r, kind="ExternalInput").ap()
    bias_d = nc.dram_tensor("bias", [128, 24], f32, kind="ExternalInput").ap()
    out_d = nc.dram_tensor("outT", [2 * HID, bs], f32, kind="ExternalOutput").ap()

    with tile.TileContext(nc) as tc:
        with (
            tc.tile_pool(name="w", bufs=1) as wpool,
            tc.tile_pool(name="x", bufs=6) as xpool,
            tc.tile_pool(name="h", bufs=8) as hpool,
            tc.tile_pool(name="o", bufs=12) as opool,
            tc.tile_pool(name="ps", bufs=8, space="PSUM") as pspool,
        ):
            # one SBUF tile per (layer, k-chunk): [128, 4*128] = 4 m-blocks
            def wchunks(K, lname):
                return [wpool.tile([128, 4 * 128], f32r, name=f"w_{lname}_{k}")
                        for k in range(K)]

            wf1 = wchunks(16, "fi1")
            wt1 = wchunks(1, "ft1")
            wc1 = wchunks(4, "ci1")
            wc2 = wchunks(4, "ct1")
            wV = wchunks(8, "V")
            wT = wchunks(8, "T")
            bt = wpool.tile([128, 24], f32, name="bias_t")

            # First batch-tile x chunks, interleaved with fi1 weight chunks so
            # DMA-ring round-robin puts each pair on different queues and the
            # k-th matmul group unblocks as soon as its own pair lands.
            xk0 = [xpool.tile([128, NT], f32r, tag="x", name=f"xk_0_{k}")
                   for k in range(16)]
            nc.sync.dma_start(bt[:], bias_d[:])
            for k in range(16):
                nc.sync.dma_start(xk0[k][:], iT_d[128 * k:128 * (k + 1), 0:NT])
                nc.sync.dma_start(wf1[k][:], w_fi1_d[:, 512 * k:512 * (k + 1)])
            xt0 = xpool.tile([128, NT], f32r, tag="xt", bufs=2, name="xt_0")
            nc.sync.dma_start(xt0[:], tT_d[:, 0:NT])
            for tiles, dram in [(wt1, w_ft1_d), (wc1, w_ci1_d), (wc2, w_ct1_d),
                                (wV, w_V_d), (wT, w_T_d)]:
                for j, wtile in enumerate(tiles):
                    nc.sync.dma_start(wtile[:], dram[:, 512 * j:512 * (j + 1)])

            def mm(ps_ap, wtiles, k, m, x_ap, start, stop):
                nc.tensor.matmul(
                    ps_ap,
                    wtiles[k][:, m * 128:(m + 1) * 128],
                    x_ap,
                    start=start,
                    stop=stop,
                )

            for n in range(ntiles):
                c0 = n * NT
                # ---- i_ = relu(i @ fi1.T + b) ----
                ps = [pspool.tile([128, NT], f32, tag="ps", name=f"ps1_{n}_{_m}") for _m in range(4)]
                if n == 0:
                    xks = xk0
                else:
                    xks = []
                    for k in range(16):
                        xk = xpool.tile([128, NT], f32r, tag="x", name=f"xk_{n}_{k}")
                        nc.sync.dma_start(xk[:], iT_d[128 * k:128 * (k + 1), c0:c0 + NT])
                        xks.append(xk)
                for k in range(16):
                    for m in range(4):
                        mm(ps[m][:], wf1, k, m, xks[k][:], k == 0, k == 15)
                i_ = [hpool.tile([128, NT], f32r, tag="i_", name=f"i__{n}_{_m}") for _m in range(4)]
                for m in range(4):
                    nc.scalar.activation(i_[m][:], ps[m][:], Relu, bias=bt[:, m:m + 1])

                # ---- t_ = relu(t @ ft1.T + b) ----
                if n == 0:
                    xt = xt0
                else:
                    xt = xpool.tile([128, NT], f32r, tag="xt", bufs=2, name=f"xt_{n}")
                    nc.sync.dma_start(xt[:], tT_d[:, c0:c0 + NT])
                ps2 = [pspool.tile([128, NT], f32, tag="ps", name=f"ps2_{n}_{_m}") for _m in range(4)]
                for m in range(4):
                    mm(ps2[m][:], wt1, 0, m, xt[:], True, True)
                t_ = [hpool.tile([128, NT], f32r, tag="t_", name=f"t__{n}_{_m}") for _m in range(4)]
                for m in range(4):
                    nc.scalar.activation(t_[m][:], ps2[m][:], Relu, bias=bt[:, 4 + m:5 + m])

                # ---- v1 = relu(i_ @ ci1.T + b) ----
                ps3 = [pspool.tile([128, NT], f32, tag="ps", name=f"ps3_{n}_{_m}") for _m in range(4)]
                for k in range(4):
                    for m in range(4):
                        mm(ps3[m][:], wc1, k, m, i_[k][:], k == 0, k == 3)
                v1 = [hpool.tile([128, NT], f32r, tag="v1", name=f"v1_{n}_{_m}") for _m in range(4)]
                for m in range(4):
                    nc.scalar.activation(v1[m][:], ps3[m][:], Relu, bias=bt[:, 8 + m:9 + m])

                # ---- v2 = relu(t_ @ ct1.T + b) ----
                ps4 = [pspool.tile([128, NT], f32, tag="ps", name=f"ps4_{n}_{_m}") for _m in range(4)]
                for k in range(4):
                    for m in range(4):
                        mm(ps4[m][:], wc2, k, m, t_[k][:], k == 0, k == 3)
                v2 = [hpool.tile([128, NT], f32r, tag="v2", name=f"v2_{n}_{_m}") for _m in range(4)]
                for m in range(4):
                    nc.scalar.activation(v2[m][:], ps4[m][:], Relu, bias=bt[:, 12 + m:13 + m])

                # ---- V = [v1, i_] @ WcatV.T + bcatV ----
                psV = [pspool.tile([128, NT], f32, tag="ps", name=f"psV_{n}_{_m}") for _m in range(4)]
                for k in range(4):
                    for m in range(4):
                        mm(psV[m][:], wV, k, m, v1[k][:], k == 0, False)
                for k in range(4):
                    for m in range(4):
                        mm(psV[m][:], wV, 4 + k, m, i_[k][:], False, k == 3)
                for m in range(4):
                    oV = opool.tile([128, NT], f32, tag="o", name=f"oV_{n}_{m}")
                    nc.scalar.activation(oV[:], psV[m][:], Ident, bias=bt[:, 16 + m:17 + m])
                    nc.sync.dma_start(out_d[128 * m:128 * (m + 1), c0:c0 + NT], oV[:])

                # ---- T = [v2, t_] @ WcatT.T + bcatT ----
                psT = [pspool.tile([128, NT], f32, tag="ps", name=f"psT_{n}_{_m}") for _m in range(4)]
                for k in range(4):
                    for m in range(4):
                        mm(psT[m][:], wT, k, m, v2[k][:], k == 0, False)
                for k in range(4):
                    for m in range(4):
                        mm(psT[m][:], wT, 4 + k, m, t_[k][:], False, k == 3)
                for m in range(4):
                    oT = opool.tile([128, NT], f32, tag="o", name=f"oT_{n}_{m}")
                    nc.scalar.activation(oT[:], psT[m][:], Ident, bias=bt[:, 20 + m:21 + m])
                    nc.sync.dma_start(
                        out_d[HID + 128 * m:HID + 128 * (m + 1), c0:c0 + NT], oT[:]
                    )

    nc.compile()
    return nc


def _host_pack(inp: dict):
    f8 = np.float64
    fi1_w, fi1_b = inp["fi1_w"], inp["fi1_b"]
    ft1_w, ft1_b = inp["ft1_w"], inp["ft1_b"]
    ci1_w, ci1_b = inp["ci1_w"], inp["ci1_b"]
    ct1_w, ct1_b = inp["ct1_w"], inp["ct1_b"]

    def fold(wv, bv, wo, bo, f_w, f_b):
        Wvo = wo.astype(f8) @ wv.astype(f8)
        bvo = wo.astype(f8) @ bv.astype(f8) + bo.astype(f8)
        Wcat = np.concatenate([f_w.astype(f8) @ Wvo, f_w.astype(f8)], axis=1)
        bcat = f_w.astype(f8) @ bvo + f_b.astype(f8)
        return Wcat.astype(np.float32), bcat.astype(np.float32)

    WcatV, bcatV = fold(inp["aV_wv"], inp["aV_bv"], inp["aV_wo"], inp["aV_bo"],
                        inp["fi2_w"], inp["fi2_b"])
    WcatT, bcatT = fold(inp["aT_wv"], inp["aT_bv"], inp["aT_wo"], inp["aT_bo"],
                        inp["ft2_w"], inp["ft2_b"])

    weights = {
        "w_fi1": _pack_blocks(np.ascontiguousarray(fi1_w.T), 16, 4),
        "w_ft1": _pack_blocks(np.ascontiguousarray(ft1_w.T), 1, 4),
        "w_ci1": _pack_blocks(np.ascontiguousarray(ci1_w.T), 4, 4),
        "w_ct1": _pack_blocks(np.ascontiguousarray(ct1_w.T), 4, 4),
        "w_V": _pack_blocks(np.ascontiguousarray(WcatV.T), 8, 4),
        "w_T": _pack_blocks(np.ascontiguousarray(WcatT.T), 8, 4),
    }
    cols = []
    for b in (fi1_b, ft1_b, ci1_b, ct1_b, bcatV, bcatT):
        for m in range(4):
            cols.append(b[128 * m:128 * (m + 1)])
    weights["bias"] = np.ascontiguousarray(np.stack(cols, axis=1), dtype=np.float32)
    return weights


def kernel(**inputs) -> np.ndarray:
    from concourse import bass_utils

    i = np.asarray(inputs["i"], dtype=np.float32)
    t = np.asarray(inputs["t"], dtype=np.float32)
    weights = _host_pack(inputs)

    if "nc" not in _CACHE:
        _CACHE["nc"] = _build_nc(BS)
    nc = _CACHE["nc"]

    in_maps = []
    for c in range(NCORES):
        sl = slice(c * BS, (c + 1) * BS)
        m = dict(weights)
        m["iT"] = np.ascontiguousarray(i[sl].T)
        m["tT"] = np.ascontiguousarray(t[sl].T)
        in_maps.append(m)

    res = bass_utils.run_bass_kernel_spmd(nc, in_maps, core_ids=list(range(NCORES)))

    out = np.empty((B, 2 * HID), dtype=np.float32)
    for c in range(NCORES):
        out[c * BS:(c + 1) * BS] = res.results[c]["outT"].T
    return out
